# revision 13
# baseline (speedup 1.0000x reference)
"""Trainium2 Bass kernel for the chunked-scan final-state problem.

Math: the reference's chunked scan + inter-chunk segsum reduces exactly to
    out[b, h, p, n] = sum_t exp(sum_{t'>t} A[b, t', h]) * X[b, t, h, p] * B[b, t, h, n]
(input C is unused by the reference).  Per (b, h) this is a (64 x 2048) @
(2048 x 16) matmul with a decay weight folded into B.

Distribution: data-parallel over batch, 8 batches per core, 8 cores.

Layout trick ("comb" K-tiles): contraction tile i takes t in {16q + i},
q = partition.  Then every DMA is fully contiguous (partition q reads rows
16q..16q+15: X 32KB, B 8KB, A 512B runs) and the decay suffix-sum becomes
  w[q, (i,h)] = exp( suffix_i(A_row q) + carry[q, h] )
where suffix_i is a 4-step shifted-add scan along the free dim and
carry = Lstrict^T @ row_totals is one small PE matmul over partitions.

Main matmuls (per batch, 16 K-tiles): stationary = weighted-B tile
(128 x 128 = all 8 heads), moving = X tile (128 x 512) -> PSUM (128 x 512)
accumulated over i; entry ((h'n), (h''p)).  The diagonal h'=h'' blocks are
the per-head outputs in (n, p) orientation; a DVE 32x32 blockwise
transpose + strided DMA writes them as (p, n) to DRAM.
"""

import numpy as np

import concourse.bacc as bacc
import concourse.mybir as mybir
import concourse.tile as tile
from concourse.bass_utils import run_bass_kernel_spmd
from concourse.masks import make_lower_triangular

F32 = mybir.dt.float32
NCORES = 8
NB = 8          # batches per core
T = 2048        # sequence length
NH = 8          # heads
DP = 64         # d_head
DN = 16         # d_state
NT = T // 128   # K-tiles of 128

_NC_CACHE = None


def _build():
    global _NC_CACHE
    if _NC_CACHE is not None:
        return _NC_CACHE

    nc = bacc.Bacc("TRN2", target_bir_lowering=False, debug=False)
    Xd = nc.dram_tensor("X", (NB, T, NH, DP), F32, kind="ExternalInput").ap()
    Ad = nc.dram_tensor("A", (NB, T, NH), F32, kind="ExternalInput").ap()
    Bd = nc.dram_tensor("B", (NB, T, NH, DN), F32, kind="ExternalInput").ap()
    # permuted output layout: O[b, j, e, pp, k, n] = out[b, 2j+e, 32k+pp, n]
    # (contiguous 128 B per-partition runs; host rearranges)
    Od = nc.dram_tensor("O", (NB, 4, 2, 32, 2, DN), F32, kind="ExternalOutput").ap()

    with tile.TileContext(nc) as tc:
        with (
            tc.tile_pool(name="consts", bufs=1) as cpool,
            tc.tile_pool(name="a1p", bufs=2) as apool,
            tc.tile_pool(name="scan", bufs=1) as spool,
            tc.tile_pool(name="wexp", bufs=2) as wpool,
            tc.tile_pool(name="bmat", bufs=2) as bpool,
            tc.tile_pool(name="bwp", bufs=2) as bwpool,
            tc.tile_pool(name="xmat", bufs=3) as xpool,
            tc.tile_pool(name="outs", bufs=3) as opool,
            tc.tile_pool(name="ps_carry", bufs=2, space="PSUM") as pcpool,
            tc.tile_pool(name="ps_main", bufs=4, space="PSUM") as pmpool,
        ):
            # strict lower-triangular constant: L[k, m] = 1 iff k > m
            ltri = cpool.tile([128, 128], F32)
            make_lower_triangular(nc, ltri[:], val=1.0, diag=False)

            # scan ping-pong buffers; pad cols stay zero forever
            va = spool.tile([128, 192], F32, tag="va")
            vb = spool.tile([128, 192], F32, tag="vb")
            nc.vector.memset(va[:, 120:192], 0.0)
            nc.vector.memset(vb[:, 128:192], 0.0)

            for b in range(NB):
                # ---- A load (fully contiguous 64 KB) ----
                a1 = apool.tile([128, 128], F32)
                nc.scalar.dma_start(
                    out=a1[:].rearrange("q (i h) -> q i h", i=NT),
                    in_=Ad[b].rearrange("(q i) h -> q i h", q=128),
                )

                # ---- strict suffix over i (16 groups of 8 cols) ----
                nc.vector.tensor_copy(va[:, 0:120], a1[:, 8:128])
                nc.vector.tensor_add(vb[:, 0:128], va[:, 0:128], va[:, 8:136])
                nc.vector.tensor_add(va[:, 0:128], vb[:, 0:128], vb[:, 16:144])
                nc.vector.tensor_add(vb[:, 0:128], va[:, 0:128], va[:, 32:160])
                nc.vector.tensor_add(va[:, 0:128], vb[:, 0:128], vb[:, 64:192])

                # row totals T[q, h] = strict_suffix(i=0) + A(i=0)
                tt = wpool.tile([128, 8], F32, tag="tt")
                nc.vector.tensor_add(tt[:], va[:, 0:8], a1[:, 0:8])
                # carry[q, h] = sum_{q' > q} T[q', h]  (partition-dim suffix)
                pc = pcpool.tile([128, 8], F32, tag="pc")
                nc.tensor.matmul(pc[:], ltri[:], tt[:], start=True, stop=True)

                # w = exp(within-row suffix + carry)
                wpre = wpool.tile([128, 128], F32, tag="wpre")
                nc.vector.tensor_add(
                    wpre[:].rearrange("q (i h) -> q i h", i=NT),
                    va[:, 0:128].rearrange("q (i h) -> q i h", i=NT),
                    pc[:].unsqueeze(1).broadcast_to((128, NT, 8)),
                )
                w = wpool.tile([128, 128], F32, tag="w")
                nc.scalar.activation(w[:], wpre[:], mybir.ActivationFunctionType.Exp)

                # ---- B load (contiguous) + decay weighting (broadcast over n) ----
                bt = bpool.tile([128, NT * 128], F32)
                nc.scalar.dma_start(
                    out=bt[:].rearrange("q (i h n) -> q i h n", i=NT, h=NH),
                    in_=Bd[b].rearrange("(q i) h n -> q i h n", q=128),
                )
                bw = bwpool.tile([128, NT * 128], F32)
                nc.vector.tensor_mul(
                    bw[:].rearrange("q (ih n) -> q ih n", n=DN),
                    bt[:].rearrange("q (ih n) -> q ih n", n=DN),
                    w[:].unsqueeze(2).broadcast_to((128, 128, DN)),
                )

                # ---- X load (contiguous 4 MB, 32 KB runs) ----
                xt = xpool.tile([128, NT * 512], F32)
                nc.sync.dma_start(
                    out=xt[:].rearrange("q (i h p) -> q i h p", i=NT, h=NH),
                    in_=Xd[b].rearrange("(q i) h p -> q i h p", q=128),
                )

                # ---- main matmuls: stationary Bw tile, moving X tile ----
                pm = pmpool.tile([128, 512], F32, tag="pm")
                for i in range(NT):
                    nc.tensor.matmul(
                        pm[:],
                        bw[:, i * 128 : (i + 1) * 128],
                        xt[:, i * 512 : (i + 1) * 512],
                        start=(i == 0),
                        stop=(i == NT - 1),
                    )

                # ---- blockwise transpose -> (p, n) blocks, then DMA out ----
                sb = opool.tile([128, 512], F32, tag="sb")
                nc.vector.tensor_copy(sb[:], pm[:])
                tb = opool.tile([128, 512], F32, tag="tb")
                nc.vector.transpose(tb[:], sb[:])
                for j in range(4):
                    # band j holds heads 2j (cols 32k+n) and 2j+1 (cols 64+32k+16+n)
                    view = tb[32 * j : 32 * j + 32, 128 * j : 128 * j + 128].rearrange(
                        "p (e k m) -> p e k m", e=2, k=2
                    )
                    nc.gpsimd.dma_start(out=Od[b, j, 0], in_=view[:, 0, :, 0:16])
                    nc.gpsimd.dma_start(out=Od[b, j, 1], in_=view[:, 1, :, 16:32])

    nc.compile()
    _NC_CACHE = nc
    return nc


def run(inputs, trace=False, tmpdir=None, trace_kwargs=None):
    """Run the SPMD kernel on 8 cores.  Returns (output, BassKernelResults)."""
    X = np.asarray(inputs["X"], dtype=np.float32)
    A = np.asarray(inputs["A"], dtype=np.float32)
    B = np.asarray(inputs["B"], dtype=np.float32)
    assert X.shape == (NCORES * NB, T, NH, DP), X.shape

    nc = _build()
    in_maps = []
    for c in range(NCORES):
        s = slice(c * NB, (c + 1) * NB)
        in_maps.append(
            {
                "X": np.ascontiguousarray(X[s]),
                "A": np.ascontiguousarray(A[s]),
                "B": np.ascontiguousarray(B[s]),
            }
        )
    kw = {}
    if trace:
        kw.update(trace=True, tmpdir=tmpdir, trace_kwargs=trace_kwargs or {})
    res = run_bass_kernel_spmd(nc, in_maps, core_ids=list(range(NCORES)), **kw)
    # O_dev[b, j, e, pp, k, n] = out[b, 2j+e, 32k+pp, n]
    raw = np.concatenate([res.results[c]["O"] for c in range(NCORES)], axis=0)
    out = np.ascontiguousarray(
        raw.transpose(0, 1, 2, 4, 3, 5).reshape(NCORES * NB, NH, DP, DN)
    )
    return out, res


def kernel(**inputs) -> np.ndarray:
    out, _ = run(inputs)
    return out


# revision 15
# speedup vs baseline: 1.1927x; 1.1927x over previous
"""Trainium2 Bass kernel for the chunked-scan final-state problem.

Math: the reference's chunked scan + inter-chunk segsum reduces exactly to
    out[b, h, p, n] = sum_t exp(sum_{t'>t} A[b, t', h]) * X[b, t, h, p] * B[b, t, h, n]
(input C is unused by the reference).  Per (b, h) this is a (64 x 2048) @
(2048 x 16) matmul with a decay weight folded into B.

Distribution: data-parallel over batch, 8 batches per core, 8 cores.

Layout trick ("comb" K-tiles): contraction tile i takes t in {16q + i},
q = partition.  Then every DMA is fully contiguous (partition q reads rows
16q..16q+15: X 32KB, B 8KB, A 512B runs) and the decay suffix-sum becomes
  w[q, (i,h)] = exp( suffix_i(A_row q) + carry[q, h] )
where suffix_i is a 4-step shifted-add scan along the free dim and
carry = Lstrict^T @ row_totals is one small PE matmul over partitions.

Main matmuls (per batch, 16 K-tiles): stationary = weighted-B tile
(128 x 128 = all 8 heads), moving = X tile (128 x 512) -> PSUM (128 x 512)
accumulated over i; entry ((h'n), (h''p)).  The diagonal h'=h'' blocks are
the per-head outputs in (n, p) orientation; a DVE 32x32 blockwise
transpose + strided DMA writes them as (p, n) to DRAM.
"""

import numpy as np

import concourse.bacc as bacc
import concourse.mybir as mybir
import concourse.tile as tile
from concourse.bass_utils import run_bass_kernel_spmd
from concourse.masks import make_lower_triangular

F32 = mybir.dt.float32
NCORES = 8
NB = 8          # batches per core
T = 2048        # sequence length
NH = 8          # heads
DP = 64         # d_head
DN = 16         # d_state
NT = T // 128   # K-tiles of 128

_NC_CACHE = None


def _build():
    global _NC_CACHE
    if _NC_CACHE is not None:
        return _NC_CACHE

    nc = bacc.Bacc("TRN2", target_bir_lowering=False, debug=False)
    Xd = nc.dram_tensor("X", (NB, T, NH, DP), F32, kind="ExternalInput").ap()
    Ad = nc.dram_tensor("A", (NB, T, NH), F32, kind="ExternalInput").ap()
    Bd = nc.dram_tensor("B", (NB, T, NH, DN), F32, kind="ExternalInput").ap()
    # permuted output layout: O[b, j, e, pp, k, n] = out[b, 2j+e, 32k+pp, n]
    # (contiguous 128 B per-partition runs; host rearranges)
    Od = nc.dram_tensor("O", (NB, 4, 2, 32, 2, DN), F32, kind="ExternalOutput").ap()

    with tile.TileContext(nc) as tc:
        with (
            tc.tile_pool(name="consts", bufs=1) as cpool,
            tc.tile_pool(name="a1p", bufs=3) as apool,
            tc.tile_pool(name="scan", bufs=1) as spool,
            tc.tile_pool(name="wexp", bufs=3) as wpool,
            tc.tile_pool(name="bmat", bufs=3) as bpool,
            tc.tile_pool(name="bwp", bufs=3) as bwpool,
            tc.tile_pool(name="xmat", bufs=3) as xpool,
            tc.tile_pool(name="outs", bufs=3) as opool,
            tc.tile_pool(name="ps_carry", bufs=2, space="PSUM") as pcpool,
            tc.tile_pool(name="ps_main", bufs=4, space="PSUM") as pmpool,
        ):
            # strict lower-triangular constant: L[k, m] = 1 iff k > m
            ltri = cpool.tile([128, 128], F32)
            make_lower_triangular(nc, ltri[:], val=1.0, diag=False)

            # scan ping-pong buffers; pad cols stay zero forever
            va = spool.tile([128, 192], F32, tag="va")
            vb = spool.tile([128, 192], F32, tag="vb")
            nc.vector.memset(va[:, 120:192], 0.0)
            nc.vector.memset(vb[:, 128:192], 0.0)

            HT = NT // 2  # tiles per X half

            def prep(b):
                """Emit loads + decay-weight chain + B weighting for batch b."""
                # ---- A load (fully contiguous 64 KB) ----
                a1 = apool.tile([128, 128], F32)
                nc.scalar.dma_start(
                    out=a1[:].rearrange("q (i h) -> q i h", i=NT),
                    in_=Ad[b].rearrange("(q i) h -> q i h", q=128),
                )

                # ---- B load (contiguous 8 KB runs) ----
                bt = bpool.tile([128, NT * 128], F32)
                nc.scalar.dma_start(
                    out=bt[:].rearrange("q (i h n) -> q i h n", i=NT, h=NH),
                    in_=Bd[b].rearrange("(q i) h n -> q i h n", q=128),
                )

                # ---- X load (two 2 MB halves, 16 KB runs) ----
                xs = []
                xr = Xd[b].rearrange("(q i) h p -> q i h p", q=128)
                for half in range(2):
                    xt = xpool.tile([128, HT * 512], F32, tag=f"x{half}")
                    nc.sync.dma_start(
                        out=xt[:].rearrange("q (i h p) -> q i h p", i=HT, h=NH),
                        in_=xr[:, half * HT : (half + 1) * HT],
                    )
                    xs.append(xt)

                # ---- strict suffix over i (16 groups of 8 cols) ----
                nc.vector.tensor_copy(va[:, 0:120], a1[:, 8:128])
                nc.vector.tensor_add(vb[:, 0:128], va[:, 0:128], va[:, 8:136])
                nc.vector.tensor_add(va[:, 0:128], vb[:, 0:128], vb[:, 16:144])
                nc.vector.tensor_add(vb[:, 0:128], va[:, 0:128], va[:, 32:160])
                nc.vector.tensor_add(va[:, 0:128], vb[:, 0:128], vb[:, 64:192])

                # row totals T[q, h] = strict_suffix(i=0) + A(i=0)
                tt = wpool.tile([128, 8], F32, tag="tt")
                nc.vector.tensor_add(tt[:], va[:, 0:8], a1[:, 0:8])
                # carry[q, h] = sum_{q' > q} T[q', h]  (partition-dim suffix)
                pc = pcpool.tile([128, 8], F32, tag="pc")
                nc.tensor.matmul(pc[:], ltri[:], tt[:], start=True, stop=True)

                # w = exp(within-row suffix + carry)
                wpre = wpool.tile([128, 128], F32, tag="wpre")
                nc.vector.tensor_add(
                    wpre[:].rearrange("q (i h) -> q i h", i=NT),
                    va[:, 0:128].rearrange("q (i h) -> q i h", i=NT),
                    pc[:].unsqueeze(1).broadcast_to((128, NT, 8)),
                )
                w = wpool.tile([128, 128], F32, tag="w")
                nc.scalar.activation(w[:], wpre[:], mybir.ActivationFunctionType.Exp)

                # ---- decay weighting of B (broadcast over n) ----
                bw = bwpool.tile([128, NT * 128], F32)
                nc.vector.tensor_mul(
                    bw[:].rearrange("q (ih n) -> q ih n", n=DN),
                    bt[:].rearrange("q (ih n) -> q ih n", n=DN),
                    w[:].unsqueeze(2).broadcast_to((128, 128, DN)),
                )
                return bw, xs

            def mains(b, bw, xs):
                """16 accumulating matmuls: stationary Bw slice, moving X slice."""
                pm = pmpool.tile([128, 512], F32, tag="pm")
                for i in range(NT):
                    xt = xs[i // HT]
                    ii = i % HT
                    nc.tensor.matmul(
                        pm[:],
                        bw[:, i * 128 : (i + 1) * 128],
                        xt[:, ii * 512 : (ii + 1) * 512],
                        start=(i == 0),
                        stop=(i == NT - 1),
                    )
                return pm

            def outs(b, pm):
                """Blockwise transpose -> per-head (p, n) blocks -> DRAM."""
                sb = opool.tile([128, 512], F32, tag="sb")
                nc.vector.tensor_copy(sb[:], pm[:])
                tb = opool.tile([128, 512], F32, tag="tb")
                nc.vector.transpose(tb[:], sb[:])
                for j in range(4):
                    # band j holds heads 2j (cols 32k+n) and 2j+1 (cols 64+32k+16+n)
                    view = tb[32 * j : 32 * j + 32, 128 * j : 128 * j + 128].rearrange(
                        "p (e k m) -> p e k m", e=2, k=2
                    )
                    nc.gpsimd.dma_start(out=Od[b, j, 0], in_=view[:, 0, :, 0:16])
                    nc.gpsimd.dma_start(out=Od[b, j, 1], in_=view[:, 1, :, 16:32])

            # software pipeline: keep DVE prep for b+1/b+2 ahead of b's outputs
            tiles = {b: prep(b) for b in range(2)}
            for b in range(NB):
                bw, xs = tiles.pop(b)
                pm = mains(b, bw, xs)
                if b + 2 < NB:
                    tiles[b + 2] = prep(b + 2)
                outs(b, pm)

    nc.compile()
    _NC_CACHE = nc
    return nc


def run(inputs, trace=False, tmpdir=None, trace_kwargs=None):
    """Run the SPMD kernel on 8 cores.  Returns (output, BassKernelResults)."""
    X = np.asarray(inputs["X"], dtype=np.float32)
    A = np.asarray(inputs["A"], dtype=np.float32)
    B = np.asarray(inputs["B"], dtype=np.float32)
    assert X.shape == (NCORES * NB, T, NH, DP), X.shape

    nc = _build()
    in_maps = []
    for c in range(NCORES):
        s = slice(c * NB, (c + 1) * NB)
        in_maps.append(
            {
                "X": np.ascontiguousarray(X[s]),
                "A": np.ascontiguousarray(A[s]),
                "B": np.ascontiguousarray(B[s]),
            }
        )
    kw = {}
    if trace:
        kw.update(trace=True, tmpdir=tmpdir, trace_kwargs=trace_kwargs or {})
    res = run_bass_kernel_spmd(nc, in_maps, core_ids=list(range(NCORES)), **kw)
    # O_dev[b, j, e, pp, k, n] = out[b, 2j+e, 32k+pp, n]
    raw = np.concatenate([res.results[c]["O"] for c in range(NCORES)], axis=0)
    out = np.ascontiguousarray(
        raw.transpose(0, 1, 2, 4, 3, 5).reshape(NCORES * NB, NH, DP, DN)
    )
    return out, res


def kernel(**inputs) -> np.ndarray:
    out, _ = run(inputs)
    return out
